# revision 3
# baseline (speedup 1.0000x reference)
"""AttnDecoderRNN single-step on 8 Trainium2 NeuronCores.

Strategy (tensor-parallel, batch=1 mat-vecs):
  Every Linear y = x @ W.T is computed on the TensorEngine with the small
  activation vector as the stationary operand (lhsT columns) and the weight
  shard streamed as the moving operand. Weight shards are pre-transposed and
  row-permuted on the host so that
    - each DMA is one contiguous [128, F] tile, and
    - lhsT k-blocks are plain columns of a row-major-loaded [128, nb] vector
      tile (vector element v lives at partition v//nb, column v%nb).
  Output dims are sharded 8-way (column-parallel); small AllGather/AllReduce
  collectives stitch the activation vectors back together between stages.
  The big c5 (V x V) stream is dependency-free and is interleaved with the
  serial attention->GRU chain to keep DMA saturated.
"""

import numpy as np

import concourse.bass as bass
import concourse.mybir as mybir
import concourse.tile as tile
from concourse import bacc
from concourse.bass_utils import run_bass_kernel_spmd

FP = mybir.dt.float32
AF = mybir.ActivationFunctionType
OP = mybir.AluOpType

NCORES = 8
H = 2048
V = 8192
ML = 4096
L = 64

# per-core output-shard sizes
N_ATTN = ML // NCORES      # 512
N_COMB = H // NCORES       # 256
N_GATE = H // NCORES       # 256  (per gate)
N_OUT = V // NCORES        # 1024
N_C5 = V // NCORES         # 1024

_ROW_BUFS = {2048: 2, 1024: 5, 768: 4, 512: 3, 256: 8}


def _build():
    nc = bacc.Bacc(num_devices=NCORES, target_bir_lowering=False, debug=False)

    I32 = mybir.dt.int32
    toks = nc.dram_tensor("toks", [L], I32, kind="ExternalInput")
    emb = nc.dram_tensor("emb", [V, H], FP, kind="ExternalInput")
    h0 = nc.dram_tensor("h0", [H], FP, kind="ExternalInput")
    h0c = nc.dram_tensor("h0c", [N_GATE], FP, kind="ExternalInput")
    hist = nc.dram_tensor("hist", [V], FP, kind="ExternalInput")
    histc = nc.dram_tensor("histc", [N_C5], FP, kind="ExternalInput")
    attn_b = nc.dram_tensor("attn_b", [N_ATTN], FP, kind="ExternalInput")
    comb_b = nc.dram_tensor("comb_b", [N_COMB], FP, kind="ExternalInput")
    bih = nc.dram_tensor("bih", [3 * N_GATE], FP, kind="ExternalInput")
    bhh = nc.dram_tensor("bhh", [3 * N_GATE], FP, kind="ExternalInput")
    out_b = nc.dram_tensor("out_b", [N_OUT], FP, kind="ExternalInput")
    c5_b = nc.dram_tensor("c5_b", [N_C5], FP, kind="ExternalInput")
    # pre-packed weight shards: [n_dma, 128, free]
    attn_w = nc.dram_tensor("attn_w", [16, 128, 1024], FP, kind="ExternalInput")
    enc_w = nc.dram_tensor("enc_w", [4, 128, 2048], FP, kind="ExternalInput")
    comb_w = nc.dram_tensor("comb_w", [8, 128, 1024], FP, kind="ExternalInput")
    wih_w = nc.dram_tensor("wih_w", [8, 128, 1536], FP, kind="ExternalInput")
    whh_w = nc.dram_tensor("whh_w", [8, 128, 1536], FP, kind="ExternalInput")
    out_w = nc.dram_tensor("out_w", [16, 128, 1024], FP, kind="ExternalInput")
    c5_w = nc.dram_tensor("c5_w", [32, 128, 2048], FP, kind="ExternalInput")

    out_sm = nc.dram_tensor("out_softmax", [1, V], FP, kind="ExternalOutput")
    out_h = nc.dram_tensor("out_hnew", [1, 1, H], FP, kind="ExternalOutput")
    out_aw = nc.dram_tensor("out_attnw", [1, ML], FP, kind="ExternalOutput")

    GRP = [list(range(NCORES))]

    with tile.TileContext(nc) as tc:
        with (
            tc.tile_pool(name="wp", bufs=3) as wp,
            tc.tile_pool(name="ms", bufs=1) as ms,
            tc.tile_pool(name="pp", bufs=1, space="PSUM") as pp,
            tc.tile_pool(name="dr", bufs=1, space="DRAM") as dr,
        ):
            def row(n, name):
                return ms.tile([1, n], FP, tag=f"r{n}", bufs=_ROW_BUFS[n],
                               name=name)

            # ---------------- constants / small loads ----------------
            ones64 = ms.tile([64, 1], FP)
            nc.vector.memset(ones64[:], 1.0 / L)
            ones128 = ms.tile([128, 1], FP)
            nc.vector.memset(ones128[:], 1.0)
            ones_row = ms.tile([1, 128], FP)
            nc.vector.memset(ones_row[:], 1.0)

            hist_blk = ms.tile([128, 64], FP)
            nc.scalar.dma_start(hist_blk[:], hist.ap().rearrange("(p b) -> p b", b=64))

            # c5 psum accumulators (live across the whole kernel)
            ps_v0 = pp.tile([1, 512], FP, tag="c5", bufs=2)
            ps_v1 = pp.tile([1, 512], FP, tag="c5", bufs=2)

            def c5_step(s):
                wt = wp.tile([128, 2048], FP, tag="c5", bufs=2, name=f"c5w_{s}")
                nc.sync.dma_start(wt[:], c5_w.ap()[s])
                for t in range(2):
                    col = s * 2 + t
                    st = col == 0
                    sp = col == 63
                    nc.tensor.matmul(
                        ps_v0[:], lhsT=hist_blk[:, col : col + 1],
                        rhs=wt[:, t * 1024 : t * 1024 + 512], start=st, stop=sp)
                    nc.tensor.matmul(
                        ps_v1[:], lhsT=hist_blk[:, col : col + 1],
                        rhs=wt[:, t * 1024 + 512 : (t + 1) * 1024], start=st, stop=sp)

            # ---------------- embedding gather + mean ----------------
            tok_sb = ms.tile([L, 1], I32)
            nc.scalar.dma_start(tok_sb[:], toks.ap().rearrange("(p o) -> p o", o=1))
            g_sb = ms.tile([L, H], FP)
            nc.gpsimd.indirect_dma_start(
                out=g_sb[:], out_offset=None, in_=emb.ap(),
                in_offset=bass.IndirectOffsetOnAxis(ap=tok_sb[:, :1], axis=0),
            )
            emb_row = row(2048, "emb_row")
            for t in range(4):
                ps_e = pp.tile([1, 512], FP, tag="mv", bufs=4, name=f"ps_e{t}")
                nc.tensor.matmul(ps_e[:], lhsT=ones64[:, :1],
                                 rhs=g_sb[:, t * 512 : (t + 1) * 512],
                                 start=True, stop=True)
                nc.scalar.copy(emb_row[:, t * 512 : (t + 1) * 512], ps_e[:])
            scr_embed = dr.tile([H], FP)
            nc.scalar.dma_start(scr_embed[:], emb_row[:])

            # lhsT vector tile for attn: [embed | h0] as [128, 32]
            cat_blk = ms.tile([128, 32], FP)
            nc.scalar.dma_start(cat_blk[:, :16],
                                scr_embed[:].rearrange("(p b) -> p b", b=16))
            nc.scalar.dma_start(cat_blk[:, 16:],
                                h0.ap().rearrange("(p b) -> p b", b=16))

            # ---------------- attention logits + exp ----------------
            ps_attn = pp.tile([1, 512], FP, tag="mv", bufs=4)
            for s in range(16):
                wt = wp.tile([128, 1024], FP, tag="attn", bufs=3, name=f"aw_{s}")
                nc.sync.dma_start(wt[:], attn_w.ap()[s])
                for t in range(2):
                    col = s * 2 + t
                    nc.tensor.matmul(ps_attn[:], lhsT=cat_blk[:, col : col + 1],
                                     rhs=wt[:, t * 512 : (t + 1) * 512],
                                     start=(col == 0), stop=(col == 31))
                if s % 4 == 0:
                    c5_step(s // 4)  # c5 super-blocks 0..3 interleaved

            ab_sb = row(512, "ab_sb")
            nc.scalar.dma_start(ab_sb[:], attn_b.ap().rearrange("(o f) -> o f", o=1))
            l_sb = row(512, "l_sb")
            nc.vector.tensor_add(l_sb[:], ps_attn[:], ab_sb[:])
            wu_sb = row(512, "wu_sb")
            nc.scalar.activation(wu_sb[:], l_sb[:], AF.Exp)
            cc_ain = dr.tile([N_ATTN], FP)
            nc.scalar.dma_start(cc_ain[:], wu_sb[:])
            cc_aout = dr.tile([ML], FP, addr_space="Shared")
            nc.gpsimd.collective_compute(
                "AllGather", OP.bypass, replica_groups=GRP,
                ins=[cc_ain.opt()], outs=[cc_aout.opt()])

            # softmax denominator from the gathered unnormalized weights
            gth = ms.tile([128, 32], FP)
            nc.scalar.dma_start(gth[:], cc_aout[:].rearrange("(p b) -> p b", b=32))
            sums = ms.tile([128, 1], FP)
            nc.vector.reduce_sum(sums[:], gth[:], axis=mybir.AxisListType.X)
            ps_S = pp.tile([1, 1], FP, tag="sc", bufs=2)
            nc.tensor.matmul(ps_S[:], lhsT=sums[:, :1], rhs=ones128[:, :1],
                             start=True, stop=True)
            rcpS = ms.tile([1, 1], FP)
            nc.vector.reciprocal(rcpS[:], ps_S[:])

            # local unnormalized weight chunk as lhsT [128, 4]
            wu_blk = ms.tile([128, 4], FP)
            nc.scalar.dma_start(wu_blk[:], cc_ain[:].rearrange("(p b) -> p b", b=4))

            # ---------------- attn_applied partial + AllReduce ----------------
            ps_app = [pp.tile([1, 512], FP, tag="mv", bufs=4, name=f"ps_app{i}")
                      for i in range(4)]
            for b in range(4):
                wt = wp.tile([128, 2048], FP, tag="enc", bufs=2, name=f"ew_{b}")
                nc.sync.dma_start(wt[:], enc_w.ap()[b])
                for n in range(4):
                    nc.tensor.matmul(ps_app[n][:], lhsT=wu_blk[:, b : b + 1],
                                     rhs=wt[:, n * 512 : (n + 1) * 512],
                                     start=(b == 0), stop=(b == 3))
                if b % 2 == 0:
                    c5_step(4 + b // 2)  # c5 blocks 4..5

            app_row = row(2048, "app_row")
            for n in range(4):
                nc.scalar.activation(app_row[:, n * 512 : (n + 1) * 512],
                                     ps_app[n][:], AF.Copy, scale=rcpS[:, :1])
            cc_pin = dr.tile([H], FP)
            nc.scalar.dma_start(cc_pin[:], app_row[:])
            cc_pout = dr.tile([H], FP, addr_space="Shared")
            nc.gpsimd.collective_compute(
                "AllReduce", OP.add, replica_groups=GRP,
                ins=[cc_pin.opt()], outs=[cc_pout.opt()])

            # attn_weights output (identical on every core)
            S_sb = ms.tile([1, 1], FP)
            nc.scalar.copy(S_sb[:], ps_S[:])
            ps_bS = pp.tile([128, 1], FP, tag="sc", bufs=2)
            nc.tensor.matmul(ps_bS[:], lhsT=ones_row[:, :], rhs=S_sb[:, :1],
                             start=True, stop=True)
            rcp128 = ms.tile([128, 1], FP)
            nc.vector.reciprocal(rcp128[:], ps_bS[:])
            aw_t = ms.tile([128, 32], FP)
            nc.vector.tensor_scalar_mul(aw_t[:], gth[:], rcp128[:, :1])
            nc.scalar.dma_start(
                out_aw.ap().rearrange("o (p b) -> (o p) b", b=32), aw_t[:])

            # ---------------- attn_combine + relu ----------------
            cat2_blk = ms.tile([128, 32], FP)
            nc.scalar.dma_start(cat2_blk[:, :16],
                                scr_embed[:].rearrange("(p b) -> p b", b=16))
            nc.scalar.dma_start(cat2_blk[:, 16:],
                                cc_pout[:].rearrange("(p b) -> p b", b=16))
            ps_x = pp.tile([1, N_COMB], FP, tag="mv", bufs=4)
            for s in range(8):
                wt = wp.tile([128, 1024], FP, tag="comb", bufs=3, name=f"cw_{s}")
                nc.sync.dma_start(wt[:], comb_w.ap()[s])
                for t in range(4):
                    col = s * 4 + t
                    nc.tensor.matmul(ps_x[:], lhsT=cat2_blk[:, col : col + 1],
                                     rhs=wt[:, t * 256 : (t + 1) * 256],
                                     start=(col == 0), stop=(col == 31))
                if s % 4 == 0:
                    c5_step(6 + s // 4)  # c5 blocks 6..7

            cb_sb = row(256, "cb_sb")
            nc.scalar.dma_start(cb_sb[:], comb_b.ap().rearrange("(o f) -> o f", o=1))
            xb_sb = row(256, "xb_sb")
            nc.vector.tensor_add(xb_sb[:], ps_x[:], cb_sb[:])
            x_sb = row(256, "x_sb")
            nc.scalar.activation(x_sb[:], xb_sb[:], AF.Relu)
            cc_xin = dr.tile([N_COMB], FP)
            nc.scalar.dma_start(cc_xin[:], x_sb[:])
            cc_xout = dr.tile([H], FP, addr_space="Shared")
            nc.gpsimd.collective_compute(
                "AllGather", OP.bypass, replica_groups=GRP,
                ins=[cc_xin.opt()], outs=[cc_xout.opt()])

            # ---------------- GRU ----------------
            # gh = h0 @ Whh.T + bhh  (independent of x -> runs early)
            ps_gh0 = pp.tile([1, 512], FP, tag="mv", bufs=4)
            ps_gh1 = pp.tile([1, 256], FP, tag="mv", bufs=4)
            for s in range(8):
                wt = wp.tile([128, 1536], FP, tag="whh", bufs=2, name=f"hw_{s}")
                nc.sync.dma_start(wt[:], whh_w.ap()[s])
                for t in range(2):
                    col = s * 2 + t
                    st = col == 0
                    sp = col == 15
                    nc.tensor.matmul(ps_gh0[:], lhsT=cat_blk[:, 16 + col : 17 + col],
                                     rhs=wt[:, t * 768 : t * 768 + 512],
                                     start=st, stop=sp)
                    nc.tensor.matmul(ps_gh1[:], lhsT=cat_blk[:, 16 + col : 17 + col],
                                     rhs=wt[:, t * 768 + 512 : (t + 1) * 768],
                                     start=st, stop=sp)
                if s % 2 == 0:
                    c5_step(8 + s // 2)  # c5 blocks 8..11

            x_blk = ms.tile([128, 16], FP)
            nc.scalar.dma_start(x_blk[:], cc_xout[:].rearrange("(p b) -> p b", b=16))
            ps_gi0 = pp.tile([1, 512], FP, tag="mv", bufs=4)
            ps_gi1 = pp.tile([1, 256], FP, tag="mv", bufs=4)
            for s in range(8):
                wt = wp.tile([128, 1536], FP, tag="wih", bufs=2, name=f"iw_{s}")
                nc.sync.dma_start(wt[:], wih_w.ap()[s])
                for t in range(2):
                    col = s * 2 + t
                    st = col == 0
                    sp = col == 15
                    nc.tensor.matmul(ps_gi0[:], lhsT=x_blk[:, col : col + 1],
                                     rhs=wt[:, t * 768 : t * 768 + 512],
                                     start=st, stop=sp)
                    nc.tensor.matmul(ps_gi1[:], lhsT=x_blk[:, col : col + 1],
                                     rhs=wt[:, t * 768 + 512 : (t + 1) * 768],
                                     start=st, stop=sp)
                if s % 2 == 0:
                    c5_step(12 + s // 2)  # c5 blocks 12..15

            NB = 3 * N_GATE  # 768
            bih_sb = row(768, "bih_sb")
            nc.scalar.dma_start(bih_sb[:], bih.ap().rearrange("(o f) -> o f", o=1))
            bhh_sb = row(768, "bhh_sb")
            nc.scalar.dma_start(bhh_sb[:], bhh.ap().rearrange("(o f) -> o f", o=1))
            gi_sb = row(768, "gi_sb")
            nc.vector.tensor_add(gi_sb[:, :512], ps_gi0[:], bih_sb[:, :512])
            nc.vector.tensor_add(gi_sb[:, 512:], ps_gi1[:], bih_sb[:, 512:])
            gh_sb = row(768, "gh_sb")
            nc.vector.tensor_add(gh_sb[:, :512], ps_gh0[:], bhh_sb[:, :512])
            nc.vector.tensor_add(gh_sb[:, 512:], ps_gh1[:], bhh_sb[:, 512:])

            NG = N_GATE
            t_r = row(256, "t_r")
            nc.vector.tensor_add(t_r[:], gi_sb[:, :NG], gh_sb[:, :NG])
            r_sb = row(256, "r_sb")
            nc.scalar.activation(r_sb[:], t_r[:], AF.Sigmoid)
            t_z = row(256, "t_z")
            nc.vector.tensor_add(t_z[:], gi_sb[:, NG : 2 * NG], gh_sb[:, NG : 2 * NG])
            z_sb = row(256, "z_sb")
            nc.scalar.activation(z_sb[:], t_z[:], AF.Sigmoid)
            t_n = row(256, "t_n")
            nc.vector.tensor_mul(t_n[:], r_sb[:], gh_sb[:, 2 * NG :])
            t_n2 = row(256, "t_n2")
            nc.vector.tensor_add(t_n2[:], gi_sb[:, 2 * NG :], t_n[:])
            n_sb = row(256, "n_sb")
            nc.scalar.activation(n_sb[:], t_n2[:], AF.Tanh)
            h0c_sb = row(256, "h0c_sb")
            nc.scalar.dma_start(h0c_sb[:], h0c.ap().rearrange("(o f) -> o f", o=1))
            t_d = row(256, "t_d")
            nc.vector.tensor_tensor(t_d[:], h0c_sb[:], n_sb[:], OP.subtract)
            t_e = row(256, "t_e")
            nc.vector.tensor_mul(t_e[:], z_sb[:], t_d[:])
            hn_sb = row(256, "hn_sb")
            nc.vector.tensor_add(hn_sb[:], n_sb[:], t_e[:])
            cc_hin = dr.tile([NG], FP)
            nc.scalar.dma_start(cc_hin[:], hn_sb[:])
            cc_hout = dr.tile([H], FP, addr_space="Shared")
            nc.gpsimd.collective_compute(
                "AllGather", OP.bypass, replica_groups=GRP,
                ins=[cc_hin.opt()], outs=[cc_hout.opt()])
            nc.scalar.dma_start(out_h.ap().rearrange("a b c -> (a b c)"), cc_hout[:])

            # ---------------- rest of c5 ----------------
            for s in range(16, 32):
                c5_step(s)

            # ---------------- out projection ----------------
            hn_blk = ms.tile([128, 16], FP)
            nc.scalar.dma_start(hn_blk[:], cc_hout[:].rearrange("(p b) -> p b", b=16))
            ps_l0 = pp.tile([1, 512], FP, tag="mv", bufs=4)
            ps_l1 = pp.tile([1, 512], FP, tag="mv", bufs=4)
            for s in range(16):
                wt = wp.tile([128, 1024], FP, tag="outw", bufs=3, name=f"ow_{s}")
                nc.sync.dma_start(wt[:], out_w.ap()[s])
                col = s
                st = col == 0
                sp = col == 15
                nc.tensor.matmul(ps_l0[:], lhsT=hn_blk[:, col : col + 1],
                                 rhs=wt[:, 0:512], start=st, stop=sp)
                nc.tensor.matmul(ps_l1[:], lhsT=hn_blk[:, col : col + 1],
                                 rhs=wt[:, 512:1024], start=st, stop=sp)

            ob_sb = row(1024, "ob_sb")
            nc.scalar.dma_start(ob_sb[:], out_b.ap().rearrange("(o f) -> o f", o=1))
            lin_sb = row(1024, "lin_sb")
            nc.vector.tensor_add(lin_sb[:, :512], ps_l0[:], ob_sb[:, :512])
            nc.vector.tensor_add(lin_sb[:, 512:], ps_l1[:], ob_sb[:, 512:])

            # ---------------- history blend ----------------
            c5b_sb = row(1024, "c5b_sb")
            nc.scalar.dma_start(c5b_sb[:], c5_b.ap().rearrange("(o f) -> o f", o=1))
            val_sb = row(1024, "val_sb")
            nc.vector.tensor_add(val_sb[:, :512], ps_v0[:], c5b_sb[:, :512])
            nc.vector.tensor_add(val_sb[:, 512:], ps_v1[:], c5b_sb[:, 512:])
            nc.scalar.activation(val_sb[:], val_sb[:], AF.Sigmoid)

            hc_sb = row(1024, "hc_sb")
            nc.scalar.dma_start(hc_sb[:], histc.ap().rearrange("(o f) -> o f", o=1))
            res_sb = row(1024, "res_sb")
            nc.vector.tensor_scalar(res_sb[:], hc_sb[:], 0.0, None, OP.not_equal)
            # blended = lin * (1 - res*val) + hist*val
            nc.vector.tensor_mul(res_sb[:], res_sb[:], val_sb[:])
            nc.scalar.activation(res_sb[:], res_sb[:], AF.Identity,
                                 bias=1.0, scale=-1.0)
            nc.vector.tensor_mul(res_sb[:], lin_sb[:], res_sb[:])
            nc.vector.tensor_mul(hc_sb[:], hc_sb[:], val_sb[:])
            bl_sb = row(1024, "bl_sb")
            nc.vector.tensor_add(bl_sb[:], res_sb[:], hc_sb[:])
            cc_oin = dr.tile([N_C5], FP)
            nc.scalar.dma_start(cc_oin[:], bl_sb[:])
            cc_oout = dr.tile([V], FP, addr_space="Shared")
            nc.gpsimd.collective_compute(
                "AllGather", OP.bypass, replica_groups=GRP,
                ins=[cc_oin.opt()], outs=[cc_oout.opt()])

            # ---------------- final softmax over V ----------------
            go = ms.tile([128, 64], FP)
            nc.scalar.dma_start(go[:], cc_oout[:].rearrange("(p b) -> p b", b=64))
            ex = ms.tile([128, 64], FP)
            sm = ms.tile([128, 1], FP)
            nc.scalar.activation(ex[:], go[:], AF.Exp, accum_out=sm[:])
            ps_S2 = pp.tile([1, 1], FP, tag="sc", bufs=2)
            nc.tensor.matmul(ps_S2[:], lhsT=sm[:, :1], rhs=ones128[:, :1],
                             start=True, stop=True)
            S2_sb = ms.tile([1, 1], FP)
            nc.scalar.copy(S2_sb[:], ps_S2[:])
            ps_bS2 = pp.tile([128, 1], FP, tag="sc", bufs=2)
            nc.tensor.matmul(ps_bS2[:], lhsT=ones_row[:, :], rhs=S2_sb[:, :1],
                             start=True, stop=True)
            rcp2 = ms.tile([128, 1], FP)
            nc.vector.reciprocal(rcp2[:], ps_bS2[:])
            outt = ms.tile([128, 64], FP)
            nc.vector.tensor_scalar_mul(outt[:], ex[:], rcp2[:, :1])
            nc.scalar.dma_start(
                out_sm.ap().rearrange("o (p b) -> (o p) b", b=64), outt[:])

    nc.compile()
    return nc


_NC_CACHE = None
_last_in_maps = None


def _get_nc():
    global _NC_CACHE
    if _NC_CACHE is None:
        _NC_CACHE = _build()
    return _NC_CACHE


def _pack(shard, per):
    """(K, n) weight shard -> [nb//per, 128, per*n] DMA tiles.

    Row k of the shard ends up in tile (k%nb)//per, partition k//nb,
    free-dim chunk (k%nb)%per -- matching lhsT column k%nb of a row-major
    [128, nb] vector tile."""
    K, n = shard.shape
    nb = K // 128
    a = shard.reshape(128, nb, n).transpose(1, 0, 2)
    a = (a.reshape(nb // per, per, 128, n).transpose(0, 2, 1, 3)
         .reshape(nb // per, 128, per * n))
    return np.ascontiguousarray(a)


def _pack_halves(shard, per):
    """Like _pack but the contraction vector is a concat of two 2048-vectors,
    each independently laid out with nb=16."""
    n = shard.shape[1]
    a1 = shard[:2048].reshape(128, 16, n).transpose(1, 0, 2)
    a2 = shard[2048:].reshape(128, 16, n).transpose(1, 0, 2)
    a = np.concatenate([a1, a2], axis=0)  # (32, 128, n)
    a = (a.reshape(32 // per, per, 128, n).transpose(0, 2, 1, 3)
         .reshape(32 // per, 128, per * n))
    return np.ascontiguousarray(a)


def kernel(input_tokens, hidden, encoder_outputs, history_record, last_hidden,
           emb, attn_W, attn_b, comb_W, comb_b,
           gru_Wih, gru_Whh, gru_bih, gru_bhh,
           out_W, out_b, c5_W, c5_b):
    f32 = lambda x: np.ascontiguousarray(np.asarray(x), dtype=np.float32)
    toks = np.ascontiguousarray(np.asarray(input_tokens), dtype=np.int32)
    emb_np = f32(emb)
    h0_np = f32(hidden).reshape(-1)          # (2048,)
    enc_np = f32(encoder_outputs)            # (4096, 2048)
    hist_np = f32(history_record).reshape(-1)

    attn_WT = f32(attn_W).T                  # (4096, 4096)
    comb_WT = f32(comb_W).T                  # (4096, 2048)
    wih_T = f32(gru_Wih).T                   # (2048, 6144)
    whh_T = f32(gru_Whh).T
    out_WT = f32(out_W).T                    # (2048, 8192)
    c5_WT = f32(c5_W).T                      # (8192, 8192)
    attn_b_np = f32(attn_b)
    comb_b_np = f32(comb_b)
    bih_np = f32(gru_bih)
    bhh_np = f32(gru_bhh)
    out_b_np = f32(out_b)
    c5_b_np = f32(c5_b)

    N_A, N_C, N_O, N_G = ML // NCORES, H // NCORES, V // NCORES, H // NCORES

    in_maps = []
    for c in range(NCORES):
        sl_a = slice(c * N_A, (c + 1) * N_A)
        sl_c = slice(c * N_C, (c + 1) * N_C)
        sl_o = slice(c * N_O, (c + 1) * N_O)
        gsl = [slice(g * H + c * N_G, g * H + (c + 1) * N_G) for g in range(3)]
        wih_c = np.concatenate([wih_T[:, s] for s in gsl], axis=1)  # (2048, 768)
        whh_c = np.concatenate([whh_T[:, s] for s in gsl], axis=1)
        in_maps.append({
            "toks": toks,
            "emb": emb_np,
            "h0": h0_np,
            "h0c": np.ascontiguousarray(h0_np[sl_c]),
            "hist": hist_np,
            "histc": np.ascontiguousarray(hist_np[sl_o]),
            "attn_b": np.ascontiguousarray(attn_b_np[sl_a]),
            "comb_b": np.ascontiguousarray(comb_b_np[sl_c]),
            "bih": np.concatenate([bih_np[s] for s in gsl]),
            "bhh": np.concatenate([bhh_np[s] for s in gsl]),
            "out_b": np.ascontiguousarray(out_b_np[sl_o]),
            "c5_b": np.ascontiguousarray(c5_b_np[sl_o]),
            "attn_w": _pack_halves(np.ascontiguousarray(attn_WT[:, sl_a]), 2),
            "enc_w": _pack(np.ascontiguousarray(enc_np[sl_a, :]), 1),
            "comb_w": _pack_halves(np.ascontiguousarray(comb_WT[:, sl_c]), 4),
            "wih_w": _pack(wih_c, 2),
            "whh_w": _pack(whh_c, 2),
            "out_w": _pack(np.ascontiguousarray(out_WT[:, sl_o]), 1),
            "c5_w": _pack(np.ascontiguousarray(c5_WT[:, sl_o]), 2),
        })

    global _last_in_maps
    _last_in_maps = in_maps
    nc = _get_nc()
    res = run_bass_kernel_spmd(nc, in_maps, list(range(NCORES))).results
    r0 = res[0]
    return (r0["out_softmax"].astype(np.float32),
            r0["out_hnew"].astype(np.float32),
            r0["out_attnw"].astype(np.float32))


# revision 4
# speedup vs baseline: 47.2292x; 47.2292x over previous
"""AttnDecoderRNN single-step on 8 Trainium2 NeuronCores.

Strategy (tensor-parallel, batch=1 mat-vecs):
  Every Linear y = x @ W.T is computed on the TensorEngine with the small
  activation vector as the stationary operand (lhsT columns) and the weight
  shard streamed as the moving operand. Weight shards are pre-transposed and
  row-permuted on the host so that
    - each DMA is one contiguous [128, F] tile, and
    - lhsT k-blocks are plain columns of a row-major-loaded [128, nb] vector
      tile (vector element v lives at partition v//nb, column v%nb).
  Output dims are sharded 8-way (column-parallel); small AllGather/AllReduce
  collectives stitch the activation vectors back together between stages.
  The big c5 (V x V) stream is dependency-free and is interleaved with the
  serial attention->GRU chain to keep DMA saturated.
"""

import numpy as np

import concourse.bass as bass
import concourse.mybir as mybir
import concourse.tile as tile
from concourse import bacc
from concourse.bass_utils import run_bass_kernel_spmd

FP = mybir.dt.float32
AF = mybir.ActivationFunctionType
OP = mybir.AluOpType

NCORES = 8
H = 2048
V = 8192
ML = 4096
L = 64

# per-core output-shard sizes
N_ATTN = ML // NCORES      # 512
N_COMB = H // NCORES       # 256
N_GATE = H // NCORES       # 256  (per gate)
N_OUT = V // NCORES        # 1024
N_C5 = V // NCORES         # 1024

_ROW_BUFS = {2048: 2, 1024: 5, 768: 4, 512: 3, 256: 8}


def _build():
    nc = bacc.Bacc(num_devices=NCORES, target_bir_lowering=False, debug=False)

    I32 = mybir.dt.int32
    toks = nc.dram_tensor("toks", [L], I32, kind="ExternalInput")
    emb = nc.dram_tensor("emb", [V, H], FP, kind="ExternalInput")
    h0 = nc.dram_tensor("h0", [H], FP, kind="ExternalInput")
    h0c = nc.dram_tensor("h0c", [N_GATE], FP, kind="ExternalInput")
    hist = nc.dram_tensor("hist", [V], FP, kind="ExternalInput")
    histc = nc.dram_tensor("histc", [N_C5], FP, kind="ExternalInput")
    attn_b = nc.dram_tensor("attn_b", [N_ATTN], FP, kind="ExternalInput")
    comb_b = nc.dram_tensor("comb_b", [N_COMB], FP, kind="ExternalInput")
    bih = nc.dram_tensor("bih", [3 * N_GATE], FP, kind="ExternalInput")
    bhh = nc.dram_tensor("bhh", [3 * N_GATE], FP, kind="ExternalInput")
    out_b = nc.dram_tensor("out_b", [N_OUT], FP, kind="ExternalInput")
    c5_b = nc.dram_tensor("c5_b", [N_C5], FP, kind="ExternalInput")
    # pre-packed weight shards: [n_dma, 128, free]
    attn_w = nc.dram_tensor("attn_w", [16, 128, 1024], FP, kind="ExternalInput")
    enc_w = nc.dram_tensor("enc_w", [4, 128, 2048], FP, kind="ExternalInput")
    comb_w = nc.dram_tensor("comb_w", [8, 128, 1024], FP, kind="ExternalInput")
    wih_w = nc.dram_tensor("wih_w", [8, 128, 1536], FP, kind="ExternalInput")
    whh_w = nc.dram_tensor("whh_w", [8, 128, 1536], FP, kind="ExternalInput")
    out_w = nc.dram_tensor("out_w", [16, 128, 1024], FP, kind="ExternalInput")
    c5_w = nc.dram_tensor("c5_w", [32, 128, 2048], FP, kind="ExternalInput")

    out_sm = nc.dram_tensor("out_softmax", [1, V], FP, kind="ExternalOutput")
    out_h = nc.dram_tensor("out_hnew", [1, 1, H], FP, kind="ExternalOutput")
    out_aw = nc.dram_tensor("out_attnw", [1, ML], FP, kind="ExternalOutput")

    GRP = [list(range(NCORES))]

    with tile.TileContext(nc) as tc:
        with (
            tc.tile_pool(name="wp", bufs=3) as wp,
            tc.tile_pool(name="ms", bufs=1) as ms,
            tc.tile_pool(name="pp", bufs=1, space="PSUM") as pp,
            tc.tile_pool(name="dr", bufs=1, space="DRAM") as dr,
        ):
            def row(n, name):
                return ms.tile([1, n], FP, tag=f"r{n}", bufs=_ROW_BUFS[n],
                               name=name)

            # ---------------- constants / small loads ----------------
            ones64 = ms.tile([64, 1], FP)
            nc.vector.memset(ones64[:], 1.0 / L)
            ones128 = ms.tile([128, 1], FP)
            nc.vector.memset(ones128[:], 1.0)
            ones_row = ms.tile([1, 128], FP)
            nc.vector.memset(ones_row[:], 1.0)

            hist_blk = ms.tile([128, 64], FP)
            nc.scalar.dma_start(hist_blk[:], hist.ap().rearrange("(p b) -> p b", b=64))

            # c5 psum accumulators (live across the whole kernel)
            ps_v0 = pp.tile([1, 512], FP, tag="c5", bufs=2)
            ps_v1 = pp.tile([1, 512], FP, tag="c5", bufs=2)

            def c5_step(s):
                wt = wp.tile([128, 2048], FP, tag="c5", bufs=2, name=f"c5w_{s}")
                nc.sync.dma_start(wt[:], c5_w.ap()[s])
                for t in range(2):
                    col = s * 2 + t
                    st = col == 0
                    sp = col == 63
                    nc.tensor.matmul(
                        ps_v0[:], lhsT=hist_blk[:, col : col + 1],
                        rhs=wt[:, t * 1024 : t * 1024 + 512], start=st, stop=sp)
                    nc.tensor.matmul(
                        ps_v1[:], lhsT=hist_blk[:, col : col + 1],
                        rhs=wt[:, t * 1024 + 512 : (t + 1) * 1024], start=st, stop=sp)

            # ---------------- embedding gather + mean ----------------
            tok_sb = ms.tile([L, 1], I32)
            nc.scalar.dma_start(tok_sb[:], toks.ap().rearrange("(p o) -> p o", o=1))
            g_sb = ms.tile([L, H], FP)
            nc.gpsimd.indirect_dma_start(
                out=g_sb[:], out_offset=None, in_=emb.ap(),
                in_offset=bass.IndirectOffsetOnAxis(ap=tok_sb[:, :1], axis=0),
            )
            emb_row = row(2048, "emb_row")
            for t in range(4):
                ps_e = pp.tile([1, 512], FP, tag="mv", bufs=4, name=f"ps_e{t}")
                nc.tensor.matmul(ps_e[:], lhsT=ones64[:, :1],
                                 rhs=g_sb[:, t * 512 : (t + 1) * 512],
                                 start=True, stop=True)
                nc.scalar.copy(emb_row[:, t * 512 : (t + 1) * 512], ps_e[:])
            scr_embed = dr.tile([H], FP)
            nc.scalar.dma_start(scr_embed[:], emb_row[:])

            # lhsT vector tile for attn: [embed | h0] as [128, 32]
            cat_blk = ms.tile([128, 32], FP)
            nc.scalar.dma_start(cat_blk[:, :16],
                                scr_embed[:].rearrange("(p b) -> p b", b=16))
            nc.scalar.dma_start(cat_blk[:, 16:],
                                h0.ap().rearrange("(p b) -> p b", b=16))

            # ---------------- attention logits + exp ----------------
            ps_attn = pp.tile([1, 512], FP, tag="mv", bufs=4)
            for s in range(16):
                wt = wp.tile([128, 1024], FP, tag="attn", bufs=3, name=f"aw_{s}")
                nc.sync.dma_start(wt[:], attn_w.ap()[s])
                for t in range(2):
                    col = s * 2 + t
                    nc.tensor.matmul(ps_attn[:], lhsT=cat_blk[:, col : col + 1],
                                     rhs=wt[:, t * 512 : (t + 1) * 512],
                                     start=(col == 0), stop=(col == 31))
                if s % 4 == 0:
                    c5_step(s // 4)  # c5 super-blocks 0..3 interleaved

            ab_sb = row(512, "ab_sb")
            nc.scalar.dma_start(ab_sb[:], attn_b.ap().rearrange("(o f) -> o f", o=1))
            l_sb = row(512, "l_sb")
            nc.vector.tensor_add(l_sb[:], ps_attn[:], ab_sb[:])
            wu_sb = row(512, "wu_sb")
            nc.scalar.activation(wu_sb[:], l_sb[:], AF.Exp)
            cc_ain = dr.tile([N_ATTN], FP)
            nc.scalar.dma_start(cc_ain[:], wu_sb[:])
            cc_aout = dr.tile([ML], FP, addr_space="Shared")
            nc.gpsimd.collective_compute(
                "AllGather", OP.bypass, replica_groups=GRP,
                ins=[cc_ain.opt()], outs=[cc_aout.opt()])

            # softmax denominator from the gathered unnormalized weights
            gth = ms.tile([128, 32], FP)
            nc.scalar.dma_start(gth[:], cc_aout[:].rearrange("(p b) -> p b", b=32))
            sums = ms.tile([128, 1], FP)
            nc.vector.reduce_sum(sums[:], gth[:], axis=mybir.AxisListType.X)
            ps_S = pp.tile([1, 1], FP, tag="sc", bufs=2)
            nc.tensor.matmul(ps_S[:], lhsT=sums[:, :1], rhs=ones128[:, :1],
                             start=True, stop=True)
            rcpS = ms.tile([1, 1], FP)
            nc.vector.reciprocal(rcpS[:], ps_S[:])

            # local unnormalized weight chunk as lhsT [128, 4]
            wu_blk = ms.tile([128, 4], FP)
            nc.scalar.dma_start(wu_blk[:], cc_ain[:].rearrange("(p b) -> p b", b=4))

            # ---------------- attn_applied partial + AllReduce ----------------
            ps_app = [pp.tile([1, 512], FP, tag="mv", bufs=4, name=f"ps_app{i}")
                      for i in range(4)]
            for b in range(4):
                wt = wp.tile([128, 2048], FP, tag="enc", bufs=2, name=f"ew_{b}")
                nc.sync.dma_start(wt[:], enc_w.ap()[b])
                for n in range(4):
                    nc.tensor.matmul(ps_app[n][:], lhsT=wu_blk[:, b : b + 1],
                                     rhs=wt[:, n * 512 : (n + 1) * 512],
                                     start=(b == 0), stop=(b == 3))
                if b % 2 == 0:
                    c5_step(4 + b // 2)  # c5 blocks 4..5

            app_row = row(2048, "app_row")
            for n in range(4):
                nc.scalar.activation(app_row[:, n * 512 : (n + 1) * 512],
                                     ps_app[n][:], AF.Copy, scale=rcpS[:, :1])
            cc_pin = dr.tile([H], FP)
            nc.scalar.dma_start(cc_pin[:], app_row[:])
            cc_pout = dr.tile([H], FP, addr_space="Shared")
            nc.gpsimd.collective_compute(
                "AllReduce", OP.add, replica_groups=GRP,
                ins=[cc_pin.opt()], outs=[cc_pout.opt()])

            # attn_weights output (identical on every core)
            S_sb = ms.tile([1, 1], FP)
            nc.scalar.copy(S_sb[:], ps_S[:])
            ps_bS = pp.tile([128, 1], FP, tag="sc", bufs=2)
            nc.tensor.matmul(ps_bS[:], lhsT=ones_row[:, :], rhs=S_sb[:, :1],
                             start=True, stop=True)
            rcp128 = ms.tile([128, 1], FP)
            nc.vector.reciprocal(rcp128[:], ps_bS[:])
            aw_t = ms.tile([128, 32], FP)
            nc.vector.tensor_scalar_mul(aw_t[:], gth[:], rcp128[:, :1])
            nc.scalar.dma_start(
                out_aw.ap().rearrange("o (p b) -> (o p) b", b=32), aw_t[:])

            # ---------------- attn_combine + relu ----------------
            cat2_blk = ms.tile([128, 32], FP)
            nc.scalar.dma_start(cat2_blk[:, :16],
                                scr_embed[:].rearrange("(p b) -> p b", b=16))
            nc.scalar.dma_start(cat2_blk[:, 16:],
                                cc_pout[:].rearrange("(p b) -> p b", b=16))
            ps_x = pp.tile([1, N_COMB], FP, tag="mv", bufs=4)
            for s in range(8):
                wt = wp.tile([128, 1024], FP, tag="comb", bufs=3, name=f"cw_{s}")
                nc.sync.dma_start(wt[:], comb_w.ap()[s])
                for t in range(4):
                    col = s * 4 + t
                    nc.tensor.matmul(ps_x[:], lhsT=cat2_blk[:, col : col + 1],
                                     rhs=wt[:, t * 256 : (t + 1) * 256],
                                     start=(col == 0), stop=(col == 31))
                if s % 4 == 0:
                    c5_step(6 + s // 4)  # c5 blocks 6..7

            cb_sb = row(256, "cb_sb")
            nc.scalar.dma_start(cb_sb[:], comb_b.ap().rearrange("(o f) -> o f", o=1))
            xb_sb = row(256, "xb_sb")
            nc.vector.tensor_add(xb_sb[:], ps_x[:], cb_sb[:])
            x_sb = row(256, "x_sb")
            nc.scalar.activation(x_sb[:], xb_sb[:], AF.Relu)
            cc_xin = dr.tile([N_COMB], FP)
            nc.scalar.dma_start(cc_xin[:], x_sb[:])
            cc_xout = dr.tile([H], FP, addr_space="Shared")
            nc.gpsimd.collective_compute(
                "AllGather", OP.bypass, replica_groups=GRP,
                ins=[cc_xin.opt()], outs=[cc_xout.opt()])

            # ---------------- GRU ----------------
            # gh = h0 @ Whh.T + bhh  (independent of x -> runs early)
            ps_gh0 = pp.tile([1, 512], FP, tag="mv", bufs=4)
            ps_gh1 = pp.tile([1, 256], FP, tag="mv", bufs=4)
            for s in range(8):
                wt = wp.tile([128, 1536], FP, tag="whh", bufs=2, name=f"hw_{s}")
                nc.sync.dma_start(wt[:], whh_w.ap()[s])
                for t in range(2):
                    col = s * 2 + t
                    st = col == 0
                    sp = col == 15
                    nc.tensor.matmul(ps_gh0[:], lhsT=cat_blk[:, 16 + col : 17 + col],
                                     rhs=wt[:, t * 768 : t * 768 + 512],
                                     start=st, stop=sp)
                    nc.tensor.matmul(ps_gh1[:], lhsT=cat_blk[:, 16 + col : 17 + col],
                                     rhs=wt[:, t * 768 + 512 : (t + 1) * 768],
                                     start=st, stop=sp)
                if s % 2 == 0:
                    c5_step(8 + s // 2)  # c5 blocks 8..11

            x_blk = ms.tile([128, 16], FP)
            nc.scalar.dma_start(x_blk[:], cc_xout[:].rearrange("(p b) -> p b", b=16))
            ps_gi0 = pp.tile([1, 512], FP, tag="mv", bufs=4)
            ps_gi1 = pp.tile([1, 256], FP, tag="mv", bufs=4)
            for s in range(8):
                wt = wp.tile([128, 1536], FP, tag="wih", bufs=2, name=f"iw_{s}")
                nc.sync.dma_start(wt[:], wih_w.ap()[s])
                for t in range(2):
                    col = s * 2 + t
                    st = col == 0
                    sp = col == 15
                    nc.tensor.matmul(ps_gi0[:], lhsT=x_blk[:, col : col + 1],
                                     rhs=wt[:, t * 768 : t * 768 + 512],
                                     start=st, stop=sp)
                    nc.tensor.matmul(ps_gi1[:], lhsT=x_blk[:, col : col + 1],
                                     rhs=wt[:, t * 768 + 512 : (t + 1) * 768],
                                     start=st, stop=sp)
                if s % 2 == 0:
                    c5_step(12 + s // 2)  # c5 blocks 12..15

            NB = 3 * N_GATE  # 768
            bih_sb = row(768, "bih_sb")
            nc.scalar.dma_start(bih_sb[:], bih.ap().rearrange("(o f) -> o f", o=1))
            bhh_sb = row(768, "bhh_sb")
            nc.scalar.dma_start(bhh_sb[:], bhh.ap().rearrange("(o f) -> o f", o=1))
            gi_sb = row(768, "gi_sb")
            nc.vector.tensor_add(gi_sb[:, :512], ps_gi0[:], bih_sb[:, :512])
            nc.vector.tensor_add(gi_sb[:, 512:], ps_gi1[:], bih_sb[:, 512:])
            gh_sb = row(768, "gh_sb")
            nc.vector.tensor_add(gh_sb[:, :512], ps_gh0[:], bhh_sb[:, :512])
            nc.vector.tensor_add(gh_sb[:, 512:], ps_gh1[:], bhh_sb[:, 512:])

            NG = N_GATE
            t_r = row(256, "t_r")
            nc.vector.tensor_add(t_r[:], gi_sb[:, :NG], gh_sb[:, :NG])
            r_sb = row(256, "r_sb")
            nc.scalar.activation(r_sb[:], t_r[:], AF.Sigmoid)
            t_z = row(256, "t_z")
            nc.vector.tensor_add(t_z[:], gi_sb[:, NG : 2 * NG], gh_sb[:, NG : 2 * NG])
            z_sb = row(256, "z_sb")
            nc.scalar.activation(z_sb[:], t_z[:], AF.Sigmoid)
            t_n = row(256, "t_n")
            nc.vector.tensor_mul(t_n[:], r_sb[:], gh_sb[:, 2 * NG :])
            t_n2 = row(256, "t_n2")
            nc.vector.tensor_add(t_n2[:], gi_sb[:, 2 * NG :], t_n[:])
            n_sb = row(256, "n_sb")
            nc.scalar.activation(n_sb[:], t_n2[:], AF.Tanh)
            h0c_sb = row(256, "h0c_sb")
            nc.scalar.dma_start(h0c_sb[:], h0c.ap().rearrange("(o f) -> o f", o=1))
            t_d = row(256, "t_d")
            nc.vector.tensor_tensor(t_d[:], h0c_sb[:], n_sb[:], OP.subtract)
            t_e = row(256, "t_e")
            nc.vector.tensor_mul(t_e[:], z_sb[:], t_d[:])
            hn_sb = row(256, "hn_sb")
            nc.vector.tensor_add(hn_sb[:], n_sb[:], t_e[:])
            cc_hin = dr.tile([NG], FP)
            nc.scalar.dma_start(cc_hin[:], hn_sb[:])
            cc_hout = dr.tile([H], FP, addr_space="Shared")
            nc.gpsimd.collective_compute(
                "AllGather", OP.bypass, replica_groups=GRP,
                ins=[cc_hin.opt()], outs=[cc_hout.opt()])
            nc.scalar.dma_start(out_h.ap().rearrange("a b c -> (a b c)"), cc_hout[:])

            # ---------------- rest of c5 ----------------
            for s in range(16, 32):
                c5_step(s)

            # ---------------- out projection ----------------
            hn_blk = ms.tile([128, 16], FP)
            nc.scalar.dma_start(hn_blk[:], cc_hout[:].rearrange("(p b) -> p b", b=16))
            ps_l0 = pp.tile([1, 512], FP, tag="mv", bufs=4)
            ps_l1 = pp.tile([1, 512], FP, tag="mv", bufs=4)
            for s in range(16):
                wt = wp.tile([128, 1024], FP, tag="outw", bufs=3, name=f"ow_{s}")
                nc.sync.dma_start(wt[:], out_w.ap()[s])
                col = s
                st = col == 0
                sp = col == 15
                nc.tensor.matmul(ps_l0[:], lhsT=hn_blk[:, col : col + 1],
                                 rhs=wt[:, 0:512], start=st, stop=sp)
                nc.tensor.matmul(ps_l1[:], lhsT=hn_blk[:, col : col + 1],
                                 rhs=wt[:, 512:1024], start=st, stop=sp)

            ob_sb = row(1024, "ob_sb")
            nc.scalar.dma_start(ob_sb[:], out_b.ap().rearrange("(o f) -> o f", o=1))
            lin_sb = row(1024, "lin_sb")
            nc.vector.tensor_add(lin_sb[:, :512], ps_l0[:], ob_sb[:, :512])
            nc.vector.tensor_add(lin_sb[:, 512:], ps_l1[:], ob_sb[:, 512:])

            # ---------------- history blend ----------------
            c5b_sb = row(1024, "c5b_sb")
            nc.scalar.dma_start(c5b_sb[:], c5_b.ap().rearrange("(o f) -> o f", o=1))
            val_sb = row(1024, "val_sb")
            nc.vector.tensor_add(val_sb[:, :512], ps_v0[:], c5b_sb[:, :512])
            nc.vector.tensor_add(val_sb[:, 512:], ps_v1[:], c5b_sb[:, 512:])
            nc.scalar.activation(val_sb[:], val_sb[:], AF.Sigmoid)

            hc_sb = row(1024, "hc_sb")
            nc.scalar.dma_start(hc_sb[:], histc.ap().rearrange("(o f) -> o f", o=1))
            res_sb = row(1024, "res_sb")
            nc.vector.tensor_scalar(res_sb[:], hc_sb[:], 0.0, None, OP.not_equal)
            # blended = lin * (1 - res*val) + hist*val
            nc.vector.tensor_mul(res_sb[:], res_sb[:], val_sb[:])
            nc.scalar.activation(res_sb[:], res_sb[:], AF.Identity,
                                 bias=1.0, scale=-1.0)
            nc.vector.tensor_mul(res_sb[:], lin_sb[:], res_sb[:])
            nc.vector.tensor_mul(hc_sb[:], hc_sb[:], val_sb[:])
            bl_sb = row(1024, "bl_sb")
            nc.vector.tensor_add(bl_sb[:], res_sb[:], hc_sb[:])
            cc_oin = dr.tile([N_C5], FP)
            nc.scalar.dma_start(cc_oin[:], bl_sb[:])
            cc_oout = dr.tile([V], FP, addr_space="Shared")
            nc.gpsimd.collective_compute(
                "AllGather", OP.bypass, replica_groups=GRP,
                ins=[cc_oin.opt()], outs=[cc_oout.opt()])

            # ---------------- final softmax over V ----------------
            go = ms.tile([128, 64], FP)
            nc.scalar.dma_start(go[:], cc_oout[:].rearrange("(p b) -> p b", b=64))
            ex = ms.tile([128, 64], FP)
            sm = ms.tile([128, 1], FP)
            nc.scalar.activation(ex[:], go[:], AF.Exp, accum_out=sm[:])
            ps_S2 = pp.tile([1, 1], FP, tag="sc", bufs=2)
            nc.tensor.matmul(ps_S2[:], lhsT=sm[:, :1], rhs=ones128[:, :1],
                             start=True, stop=True)
            S2_sb = ms.tile([1, 1], FP)
            nc.scalar.copy(S2_sb[:], ps_S2[:])
            ps_bS2 = pp.tile([128, 1], FP, tag="sc", bufs=2)
            nc.tensor.matmul(ps_bS2[:], lhsT=ones_row[:, :], rhs=S2_sb[:, :1],
                             start=True, stop=True)
            rcp2 = ms.tile([128, 1], FP)
            nc.vector.reciprocal(rcp2[:], ps_bS2[:])
            outt = ms.tile([128, 64], FP)
            nc.vector.tensor_scalar_mul(outt[:], ex[:], rcp2[:, :1])
            nc.scalar.dma_start(
                out_sm.ap().rearrange("o (p b) -> (o p) b", b=64), outt[:])

    nc.compile()
    return nc


def _build_null():
    """Same I/O signature, trivial body — for benchmark overhead calibration."""
    nc = bacc.Bacc(num_devices=NCORES, target_bir_lowering=False, debug=False)
    I32 = mybir.dt.int32
    nc.dram_tensor("toks", [L], I32, kind="ExternalInput")
    nc.dram_tensor("emb", [V, H], FP, kind="ExternalInput")
    nc.dram_tensor("h0", [H], FP, kind="ExternalInput")
    nc.dram_tensor("h0c", [N_GATE], FP, kind="ExternalInput")
    nc.dram_tensor("hist", [V], FP, kind="ExternalInput")
    nc.dram_tensor("histc", [N_C5], FP, kind="ExternalInput")
    nc.dram_tensor("attn_b", [N_ATTN], FP, kind="ExternalInput")
    nc.dram_tensor("comb_b", [N_COMB], FP, kind="ExternalInput")
    nc.dram_tensor("bih", [3 * N_GATE], FP, kind="ExternalInput")
    nc.dram_tensor("bhh", [3 * N_GATE], FP, kind="ExternalInput")
    nc.dram_tensor("out_b", [N_OUT], FP, kind="ExternalInput")
    nc.dram_tensor("c5_b", [N_C5], FP, kind="ExternalInput")
    nc.dram_tensor("attn_w", [16, 128, 1024], FP, kind="ExternalInput")
    nc.dram_tensor("enc_w", [4, 128, 2048], FP, kind="ExternalInput")
    nc.dram_tensor("comb_w", [8, 128, 1024], FP, kind="ExternalInput")
    nc.dram_tensor("wih_w", [8, 128, 1536], FP, kind="ExternalInput")
    nc.dram_tensor("whh_w", [8, 128, 1536], FP, kind="ExternalInput")
    nc.dram_tensor("out_w", [16, 128, 1024], FP, kind="ExternalInput")
    nc.dram_tensor("c5_w", [32, 128, 2048], FP, kind="ExternalInput")
    out_sm = nc.dram_tensor("out_softmax", [1, V], FP, kind="ExternalOutput")
    out_h = nc.dram_tensor("out_hnew", [1, 1, H], FP, kind="ExternalOutput")
    out_aw = nc.dram_tensor("out_attnw", [1, ML], FP, kind="ExternalOutput")
    with tile.TileContext(nc) as tc:
        with tc.tile_pool(name="sb", bufs=1) as sb:
            t = sb.tile([128, 64], FP)
            nc.vector.memset(t[:], 0.0)
            nc.scalar.dma_start(out_sm.ap().rearrange("o (p b) -> (o p) b", b=64), t[:])
            nc.scalar.dma_start(
                out_h.ap().rearrange("a b (p c) -> (a b p) c", c=16), t[:, :16])
            nc.scalar.dma_start(
                out_aw.ap().rearrange("o (p b) -> (o p) b", b=32), t[:, :32])
    nc.compile()
    return nc


_NC_CACHE = None
_last_in_maps = None


def _get_nc():
    global _NC_CACHE
    if _NC_CACHE is None:
        _NC_CACHE = _build()
    return _NC_CACHE


def _pack(shard, per):
    """(K, n) weight shard -> [nb//per, 128, per*n] DMA tiles.

    Row k of the shard ends up in tile (k%nb)//per, partition k//nb,
    free-dim chunk (k%nb)%per -- matching lhsT column k%nb of a row-major
    [128, nb] vector tile."""
    K, n = shard.shape
    nb = K // 128
    a = shard.reshape(128, nb, n).transpose(1, 0, 2)
    a = (a.reshape(nb // per, per, 128, n).transpose(0, 2, 1, 3)
         .reshape(nb // per, 128, per * n))
    return np.ascontiguousarray(a)


def _pack_halves(shard, per):
    """Like _pack but the contraction vector is a concat of two 2048-vectors,
    each independently laid out with nb=16."""
    n = shard.shape[1]
    a1 = shard[:2048].reshape(128, 16, n).transpose(1, 0, 2)
    a2 = shard[2048:].reshape(128, 16, n).transpose(1, 0, 2)
    a = np.concatenate([a1, a2], axis=0)  # (32, 128, n)
    a = (a.reshape(32 // per, per, 128, n).transpose(0, 2, 1, 3)
         .reshape(32 // per, 128, per * n))
    return np.ascontiguousarray(a)


def kernel(input_tokens, hidden, encoder_outputs, history_record, last_hidden,
           emb, attn_W, attn_b, comb_W, comb_b,
           gru_Wih, gru_Whh, gru_bih, gru_bhh,
           out_W, out_b, c5_W, c5_b):
    f32 = lambda x: np.ascontiguousarray(np.asarray(x), dtype=np.float32)
    toks = np.ascontiguousarray(np.asarray(input_tokens), dtype=np.int32)
    emb_np = f32(emb)
    h0_np = f32(hidden).reshape(-1)          # (2048,)
    enc_np = f32(encoder_outputs)            # (4096, 2048)
    hist_np = f32(history_record).reshape(-1)

    attn_WT = f32(attn_W).T                  # (4096, 4096)
    comb_WT = f32(comb_W).T                  # (4096, 2048)
    wih_T = f32(gru_Wih).T                   # (2048, 6144)
    whh_T = f32(gru_Whh).T
    out_WT = f32(out_W).T                    # (2048, 8192)
    c5_WT = f32(c5_W).T                      # (8192, 8192)
    attn_b_np = f32(attn_b)
    comb_b_np = f32(comb_b)
    bih_np = f32(gru_bih)
    bhh_np = f32(gru_bhh)
    out_b_np = f32(out_b)
    c5_b_np = f32(c5_b)

    N_A, N_C, N_O, N_G = ML // NCORES, H // NCORES, V // NCORES, H // NCORES

    in_maps = []
    for c in range(NCORES):
        sl_a = slice(c * N_A, (c + 1) * N_A)
        sl_c = slice(c * N_C, (c + 1) * N_C)
        sl_o = slice(c * N_O, (c + 1) * N_O)
        gsl = [slice(g * H + c * N_G, g * H + (c + 1) * N_G) for g in range(3)]
        wih_c = np.concatenate([wih_T[:, s] for s in gsl], axis=1)  # (2048, 768)
        whh_c = np.concatenate([whh_T[:, s] for s in gsl], axis=1)
        in_maps.append({
            "toks": toks,
            "emb": emb_np,
            "h0": h0_np,
            "h0c": np.ascontiguousarray(h0_np[sl_c]),
            "hist": hist_np,
            "histc": np.ascontiguousarray(hist_np[sl_o]),
            "attn_b": np.ascontiguousarray(attn_b_np[sl_a]),
            "comb_b": np.ascontiguousarray(comb_b_np[sl_c]),
            "bih": np.concatenate([bih_np[s] for s in gsl]),
            "bhh": np.concatenate([bhh_np[s] for s in gsl]),
            "out_b": np.ascontiguousarray(out_b_np[sl_o]),
            "c5_b": np.ascontiguousarray(c5_b_np[sl_o]),
            "attn_w": _pack_halves(np.ascontiguousarray(attn_WT[:, sl_a]), 2),
            "enc_w": _pack(np.ascontiguousarray(enc_np[sl_a, :]), 1),
            "comb_w": _pack_halves(np.ascontiguousarray(comb_WT[:, sl_c]), 4),
            "wih_w": _pack(wih_c, 2),
            "whh_w": _pack(whh_c, 2),
            "out_w": _pack(np.ascontiguousarray(out_WT[:, sl_o]), 1),
            "c5_w": _pack(np.ascontiguousarray(c5_WT[:, sl_o]), 2),
        })

    global _last_in_maps
    _last_in_maps = in_maps
    nc = _get_nc()
    res = run_bass_kernel_spmd(nc, in_maps, list(range(NCORES))).results
    r0 = res[0]
    return (r0["out_softmax"].astype(np.float32),
            r0["out_hnew"].astype(np.float32),
            r0["out_attnw"].astype(np.float32))
